# revision 18
# baseline (speedup 1.0000x reference)
"""GQA causal-attention prefill (B=2, T=S=2048, D=2048, N=16, K=4, H=128)
on 8 Trainium2 NeuronCores.

Sharding: one (batch, kv-head) pair per core -> 2*4 = 8 cores, zero
cross-core communication on device; the o_proj partial sums (over each
batch's 4 kv-head groups) are reduced on the host at unshard time.

v2 design (vs. f32r baseline at 345us):
  * all matmul operands bf16 (PSUM accumulation stays f32): removes the
    fp32r 4-cyc/row penalty at moving<256, halves LDWEIGHTS and DMA.
  * single fused chunk loop (proj c -> attention c -> o_proj c) in one
    pool scope so the Tile scheduler can overlap phases.
  * causal mask applied by a PE matmul (ident^T @ tri -> PSUM with
    start=True; the score matmul accumulates on top with start=False),
    replacing 64 slow DVE PSUM adds.
  * softmax denominator via col-tiled [128,32]-ones matmuls into one
    shared PSUM bank (4 heads at col groups 0/32/64/96); reciprocal on
    the whole bank; broadcast back via K=1 f32r matmul.
  * attention runs over head pairs: PSUM = 2 ot + 2 score + 1 den +
    3 flex (proj/rope/transpose/bcast/o_proj) = 8 banks.
  * coarse DMA: one descriptor per (chunk, tensor).
"""
import sys
import types

import numpy as np
import ml_dtypes

try:  # make trace=True degrade gracefully when axon_hooks is absent
    import antenv.axon_hooks  # noqa: F401
except Exception:
    try:
        import antenv
        _m = types.ModuleType("antenv.axon_hooks")
        _h = [None]
        _m.set_axon_ntff_profile_hook = lambda h: _h.__setitem__(0, h)
        _m.get_axon_ntff_profile_hook = lambda: _h[0]
        sys.modules["antenv.axon_hooks"] = _m
        antenv.axon_hooks = _m
    except Exception:
        pass

import concourse.bass as bass  # noqa: F401
from concourse import bacc
import concourse.tile as tile
import concourse.mybir as mybir
from concourse.bass_utils import run_bass_kernel_spmd
from concourse.masks import make_identity

B, T, D = 2, 2048, 2048
N, K, H = 16, 4, 128
G = N // K
HALF = H // 2
MIN_TS, MAX_TS = 1.0, 10000.0

P = 128
TCH = 512
NCH = T // TCH          # 4 t-chunks of 512
DB = D // P             # 16 contraction blocks
F32 = mybir.dt.float32
F32R = mybir.dt.float32r
BF16 = mybir.dt.bfloat16
FP16 = mybir.dt.float16
NEG = -1.0e30
EXP = mybir.ActivationFunctionType.Exp
BF = ml_dtypes.bfloat16

_CACHE = {}
LAST = None             # BassKernelResults of the most recent run


def _build():
    if "nc" in _CACHE:
        return _CACHE["nc"]
    nc = bacc.Bacc(None, target_bir_lowering=False, debug=False)
    xq = nc.declare_dram_parameter("xqT", [D, T], FP16, isOutput=False)
    xkv = nc.declare_dram_parameter("xkvT", [D, T], FP16, isOutput=False)
    wq = nc.declare_dram_parameter("wq", [D, G * H], FP16, isOutput=False)
    wk = nc.declare_dram_parameter("wk", [D, H], FP16, isOutput=False)
    wv = nc.declare_dram_parameter("wv", [D, H], FP16, isOutput=False)
    wo = nc.declare_dram_parameter("wo", [G, H, D], BF16, isOutput=False)
    cq = nc.declare_dram_parameter("cosq", [P, T], FP16, isOutput=False)
    sq = nc.declare_dram_parameter("sinq", [P, T], FP16, isOutput=False)
    tri = nc.declare_dram_parameter("tri", [P, P], BF16, isOutput=False)
    o32 = nc.declare_dram_parameter("ones32", [P, 32], BF16, isOutput=False)
    onef = nc.declare_dram_parameter("one_r", [1, P], F32R, isOutput=False)
    pi = nc.declare_dram_parameter("pi", [P, P], FP16, isOutput=False)
    out = nc.declare_dram_parameter("O", [T, D], F32, isOutput=True)
    import os
    DBG = bool(int(os.environ.get("KDBG", "0")))
    if DBG:
        den_o = nc.declare_dram_parameter("den_o", [NCH, P, TCH], F32, isOutput=True)
        bc_o = nc.declare_dram_parameter("bc_o", [G, P, TCH], BF16, isOutput=True)
        ot_o = nc.declare_dram_parameter("ot_o", [G, P, TCH], F32, isOutput=True)
        qt_o = nc.declare_dram_parameter("qt_o", [P, G * T], FP16, isOutput=True)
        kt_o = nc.declare_dram_parameter("kt_o", [P, T], FP16, isOutput=True)
        pb_o = nc.declare_dram_parameter("pb_o", [P, TCH], BF16, isOutput=True)
        inv_o = nc.declare_dram_parameter("inv_o", [2, 64, TCH], F32, isOutput=True)
        invb_o = nc.declare_dram_parameter("invb_o", [2, 64, TCH], F32, isOutput=True)
        vs_o = nc.declare_dram_parameter("vs_o", [P, DB * H], BF16, isOutput=True)
        otc_o = nc.declare_dram_parameter("otc_o", [NCH, P, G, TCH], BF16, isOutput=True)

    xq_v = xq[:].rearrange("(do di) t -> di do t", di=P)
    xkv_v = xkv[:].rearrange("(do di) t -> di do t", di=P)
    wq_v = wq[:].rearrange("(do di) nh -> di do nh", di=P)
    wk_v = wk[:].rearrange("(do di) h -> di do h", di=P)
    wv_v = wv[:].rearrange("(do di) h -> di do h", di=P)
    wo_v = wo[:].rearrange("n h d -> h n d")

    with tile.TileContext(nc) as tc:
        with tc.tile_pool(name="glob", bufs=1) as glob, \
             tc.tile_pool(name="xp", bufs=2) as xp, \
             tc.tile_pool(name="dstp", bufs=4) as dstp, \
             tc.tile_pool(name="prodp", bufs=4) as prodp, \
             tc.tile_pool(name="pbp", bufs=6) as pbp, \
             tc.tile_pool(name="otcp", bufs=2) as otcp, \
             tc.tile_pool(name="invp", bufs=2) as invp, \
             tc.tile_pool(name="osbp", bufs=3) as osbp, \
             tc.tile_pool(name="ps_ot", bufs=2, space="PSUM") as ps_ot, \
             tc.tile_pool(name="ps_sc", bufs=2, space="PSUM") as ps_sc, \
             tc.tile_pool(name="ps_den", bufs=2, space="PSUM") as ps_den, \
             tc.tile_pool(name="ps_fx", bufs=2, space="PSUM") as ps_fx:
            qt = glob.tile([P, G, T], FP16)
            kt = glob.tile([P, T], FP16)
            vsb = glob.tile([P, DB, H], BF16)
            wq_sb = glob.tile([P, DB, G * H], FP16)
            wk_sb = glob.tile([P, DB, H], FP16)
            wv_sb = glob.tile([P, DB, H], FP16)
            wo_sb = glob.tile([P, G, D], BF16)
            cosq_sb = glob.tile([P, T], FP16)
            sinq_sb = glob.tile([P, T], FP16)
            tri_sb = glob.tile([P, P], BF16)
            ones32_sb = glob.tile([P, 32], BF16)
            one_r = glob.tile([1, P], F32R)
            pi_sb = glob.tile([P, P], FP16)
            ident = glob.tile([P, P], BF16)
            zer = glob.tile([P, P], FP16)
            nc.gpsimd.memset(zer[:], 0.0)

            # tables/weights on the gpsimd queue; bulk x on sync queue
            nc.gpsimd.dma_start(tri_sb[:], tri[:])
            nc.gpsimd.dma_start(ones32_sb[:], o32[:])
            nc.gpsimd.dma_start(one_r[:], onef[:])
            nc.gpsimd.dma_start(pi_sb[:], pi[:])
            nc.gpsimd.dma_start(cosq_sb[:], cq[:])
            nc.gpsimd.dma_start(sinq_sb[:], sq[:])
            nc.gpsimd.dma_start(wk_sb[:], wk_v)
            nc.gpsimd.dma_start(wv_sb[:], wv_v)
            nc.gpsimd.dma_start(wo_sb[:], wo_v)
            nc.sync.dma_start(wq_sb[:], wq_v)
            make_identity(nc, ident[:])

            # warm the PE HAM while initial DMAs land
            warm_ps = ps_fx.tile([P, TCH], F32, tag="fx")
            for _ in range(24):
                nc.tensor.matmul(warm_ps[:, :P], ident[:], ident[:],
                                 start=True, stop=True)

            def rope(ps, out_sl, tsl):
                """out_sl[128,TCH] (bf16) = ps*cos + rot(ps)*sin."""
                dst = dstp.tile([P, TCH], FP16, tag="dst")
                nc.scalar.copy(dst[:], ps[:])
                rot = ps_fx.tile([P, TCH], F32, tag="fx")
                nc.tensor.matmul(rot[:], pi_sb[:], dst[:],
                                 start=True, stop=True)
                prod = prodp.tile([P, TCH], FP16, tag="prod")
                nc.vector.tensor_mul(prod[:], rot[:], sinq_sb[:, tsl])
                nc.vector.tensor_mul(out_sl, dst[:], cosq_sb[:, tsl])
                nc.vector.tensor_add(out_sl, out_sl, prod[:])

            for c in range(NCH):
                tsl = slice(c * TCH, (c + 1) * TCH)
                # ---- input DMA for this chunk (one descriptor each) ----
                xq_sb = xp.tile([P, DB, TCH], FP16, tag="xq")
                xkv_sb = xp.tile([P, DB, TCH], FP16, tag="xkv")
                nc.sync.dma_start(xq_sb[:], xq_v[:, :, tsl])
                nc.sync.dma_start(xkv_sb[:], xkv_v[:, :, tsl])

                # ---- projections + RoPE ----
                for n in range(G):
                    ps = ps_fx.tile([P, TCH], F32, tag="fx")
                    for db in range(DB):
                        nc.tensor.matmul(
                            ps[:], wq_sb[:, db, n * H:(n + 1) * H],
                            xq_sb[:, db, :],
                            start=(db == 0), stop=(db == DB - 1))
                    rope(ps, qt[:, n, tsl], tsl)
                ps = ps_fx.tile([P, TCH], F32, tag="fx")
                for db in range(DB):
                    nc.tensor.matmul(ps[:], wk_sb[:, db, :], xkv_sb[:, db, :],
                                     start=(db == 0), stop=(db == DB - 1))
                rope(ps, kt[:, tsl], tsl)
                ps = ps_fx.tile([P, TCH], F32, tag="fx")
                for db in range(DB):
                    nc.tensor.matmul(ps[:], wv_sb[:, db, :], xkv_sb[:, db, :],
                                     start=(db == 0), stop=(db == DB - 1))
                vt = dstp.tile([P, TCH], BF16, tag="vt", bufs=2)
                nc.scalar.copy(vt[:], ps[:])
                for kk in range(4):
                    pst = ps_fx.tile([P, P], BF16, tag="fx")
                    nc.tensor.transpose(pst[:], vt[:, kk * P:(kk + 1) * P],
                                        ident[:])
                    nc.scalar.copy(vsb[:, 4 * c + kk, :], pst[:])

                # ---- attention over head pairs ----
                J = 4 * (c + 1)
                otc = otcp.tile([P, G, TCH], BF16, tag="otc")
                for p in range(2):
                    heads = (2 * p, 2 * p + 1)
                    den_ps = ps_den.tile([P, TCH], F32, tag="den")
                    nc.tensor.matmul(den_ps[:], zer[:], cosq_sb[:, 0:TCH],
                                     start=True, stop=False)
                    ots = [ps_ot.tile([P, TCH], F32, tag="ot", name="ot")
                           for _ in heads]
                    for j in range(J):
                        d = j - 4 * c
                        lo = max(d, 0) * P
                        pbs = []
                        for i, h in enumerate(heads):
                            sc = ps_sc.tile([P, TCH], F32, tag="sc")
                            if d >= 0:
                                nc.tensor.matmul(
                                    sc[:, lo:lo + P], ident[:], tri_sb[:],
                                    start=True, stop=False)
                                nc.tensor.matmul(
                                    sc[:, lo:], kt[:, j * P:(j + 1) * P],
                                    qt[:, h, c * TCH + lo:(c + 1) * TCH],
                                    start=False, stop=True)
                            else:
                                nc.tensor.matmul(
                                    sc[:], kt[:, j * P:(j + 1) * P],
                                    qt[:, h, tsl],
                                    start=True, stop=True)
                            pb = pbp.tile([P, TCH], BF16, tag="pb")
                            nc.scalar.activation(pb[:, lo:], sc[:, lo:], EXP)
                            pbs.append(pb)
                            if DBG and c == 0 and p == 0 and j == 0 and i == 0:
                                nc.sync.dma_start(pb_o[:], pb[:])
                        for i, h in enumerate(heads):
                            nc.tensor.matmul(
                                ots[i][:, lo:], vsb[:, j, :], pbs[i][:, lo:],
                                start=(j == 0), stop=(j == J - 1))
                        for i, h in enumerate(heads):
                            nc.tensor.matmul(
                                den_ps[32 * i:32 * i + 32, lo:],
                                ones32_sb[:], pbs[i][:, lo:],
                                start=False,
                                stop=(j == J - 1 and i == 1),
                                tile_position=(0, 32 * i))
                    # normalize this pair (baseline-style bcast)
                    for i, h in enumerate(heads):
                        den_row = invp.tile([1, TCH], F32, tag="den_row")
                        nc.vector.tensor_copy(
                            den_row[:], den_ps[32 * i:32 * i + 1, :])
                        inv = invp.tile([1, TCH], F32, tag="inv")
                        nc.vector.reciprocal_approx_fast(
                            out=inv[:], in_=den_row[:])
                        invr = invp.tile([1, TCH], F32R, tag="invr")
                        nc.vector.tensor_copy(invr[:], inv[:])
                        if DBG and c == 0:
                            nc.sync.dma_start(inv_o[p, 32 * i], inv[:])
                            nc.gpsimd.dma_start(invb_o[p, 32 * i], invr[:])
                        bc = ps_fx.tile([P, TCH], F32, tag="fx")
                        nc.tensor.matmul(
                            bc[:], one_r[:], invr[:],
                            start=True, stop=True)
                        bcb = prodp.tile([P, TCH], BF16, tag="bcb")
                        nc.scalar.copy(bcb[:], bc[:])
                        nc.vector.tensor_mul(otc[:, h, :], ots[i][:], bcb[:])
                        if DBG and c == 0:
                            nc.sync.dma_start(bc_o[h], bcb[:])
                            otst = osbp.tile([P, TCH], F32, tag="otst", bufs=1)
                            nc.vector.tensor_copy(otst[:], ots[i][:])
                            nc.sync.dma_start(ot_o[h], otst[:])

                if DBG:
                    nc.sync.dma_start(otc_o[c], otc[:])
                    dstage = osbp.tile([P, TCH], F32, tag="dstage", bufs=1)
                    nc.vector.tensor_copy(dstage[:], den_ps[:])
                    nc.sync.dma_start(den_o[c], dstage[:])
                    if c == NCH - 1:
                        nc.sync.dma_start(qt_o[:].rearrange("p (g t) -> p g t", g=G), qt[:])
                        nc.sync.dma_start(kt_o[:], kt[:])
                        nc.sync.dma_start(vs_o[:].rearrange("p (j h) -> p j h", j=DB), vsb[:])

                # ---- o_proj for this chunk ----
                for kk in range(4):
                    row = c * TCH + kk * P
                    osb = osbp.tile([P, D], F32, tag="osb")
                    for dc in range(4):
                        ops = ps_fx.tile([P, TCH], F32, tag="fx")
                        for n in range(G):
                            nc.tensor.matmul(
                                ops[:],
                                otc[:, n, kk * P:(kk + 1) * P],
                                wo_sb[:, n, dc * TCH:(dc + 1) * TCH],
                                start=(n == 0), stop=(n == G - 1))
                        nc.scalar.copy(osb[:, dc * TCH:(dc + 1) * TCH], ops[:])
                    nc.sync.dma_start(out[row:row + P, :], osb[:])

    nc.compile()
    _CACHE["nc"] = nc
    return nc


def _rope_tables(pos):
    ts = MIN_TS * (MAX_TS / MIN_TS) ** (2.0 * np.arange(HALF) / H)
    ang = pos.astype(np.float64)[None, :] / ts[:, None]
    c, s = np.cos(ang), np.sin(ang)
    cosF = np.ascontiguousarray(np.concatenate([c, c], 0)).astype(np.float16)
    sinF = np.ascontiguousarray(np.concatenate([-s, s], 0)).astype(np.float16)
    return cosF, sinF


def kernel(Xq, Xkv, q_positions, kv_positions, Wq, Wk, Wv, Wo, _trace=False):
    global LAST
    nc = _build()
    Xq = np.asarray(Xq, dtype=np.float32)
    Xkv = np.asarray(Xkv, dtype=np.float32)
    Wq = np.asarray(Wq, dtype=np.float32)
    Wk = np.asarray(Wk, dtype=np.float32)
    Wv = np.asarray(Wv, dtype=np.float32)
    Wo = np.asarray(Wo, dtype=np.float32)
    qp = np.asarray(q_positions)
    kp = np.asarray(kv_positions)
    assert np.array_equal(qp, kp), (
        "kernel assumes q_positions == kv_positions (RoPE tables shared)")

    idx = np.arange(P)
    tri_np = np.where(idx[:, None] <= idx[None, :], 0.0, NEG).astype(BF)
    pi_np = np.zeros((P, P), np.float32)
    pi_np[(idx + HALF) % P, idx] = 1.0
    pi_np = pi_np.astype(np.float16)
    ones32_np = np.ones((P, 32), BF)
    one_r_np = np.ones((1, P), np.float32)

    xqT = [np.ascontiguousarray(Xq[b].T).astype(np.float16) for b in range(B)]
    xkvT = [np.ascontiguousarray(Xkv[b].T).astype(np.float16) for b in range(B)]
    ctabs = [_rope_tables(qp[b]) for b in range(B)]
    wqs = [np.ascontiguousarray(
        Wq[:, kv * G:(kv + 1) * G, :].reshape(D, G * H)).astype(np.float16)
        for kv in range(K)]
    wks = [np.ascontiguousarray(Wk[:, kv, :]).astype(np.float16)
           for kv in range(K)]
    wvs = [np.ascontiguousarray(Wv[:, kv, :]).astype(np.float16)
           for kv in range(K)]
    wos = [np.ascontiguousarray(Wo[kv * G:(kv + 1) * G]).astype(BF)
           for kv in range(K)]

    in_maps = []
    for core in range(8):
        b, kv = divmod(core, 4)
        in_maps.append({
            "xqT": xqT[b],
            "xkvT": xkvT[b],
            "wq": wqs[kv],
            "wk": wks[kv],
            "wv": wvs[kv],
            "wo": wos[kv],
            "cosq": ctabs[b][0], "sinq": ctabs[b][1],
            "tri": tri_np,
            "ones32": ones32_np,
            "one_r": one_r_np,
            "pi": pi_np,
        })

    LAST = run_bass_kernel_spmd(nc, in_maps, list(range(8)), trace=_trace)
    parts = [r["O"] for r in LAST.results]
    O = np.stack([parts[0] + parts[1] + parts[2] + parts[3],
                  parts[4] + parts[5] + parts[6] + parts[7]])
    return np.ascontiguousarray(O.astype(np.float32))
